# revision 26
# baseline (speedup 1.0000x reference)
"""GAT (2-layer, DGL-style) Bass kernel for Trainium2, 8-core SPMD.

Strategy (dst-sharded, edge-parallel within core):
- Node rows are sharded across 8 cores by dst range (6250 nodes/core).
- GEMMs (h = x @ W) are data-parallel over nodes; el/er attention dot
  products are folded into the GEMM by extending W with W @ attn^T cols.
- Layer-1 node data is split into TWO tables: hmsg [N, 512] bf16 (message
  rows, gathered at exactly 1024 B/edge - the power-of-two element size
  runs at full HBM rate) and hatt [N, 128] bf16 ([el 8 | er 8 | pad]
  windows, gathered at 256 B/edge, hidden under the msg stream).
- Both tables are AllGathered so each core can gather any src row.
- Each core processes all edges whose dst lands in its node range:
  per-edge rows are fetched with dma_gather round-robined over 4 SWDGE
  queues, attention weights are computed in-place, and the segment
  softmax numerator/denominator accumulate per 128-dst-node block with
  one-hot matmuls into PSUM.
- DVE ops use pair-expanded broadcast operands (innermost [stride 1,
  count 2]) so the vector engine runs in its 2x/4x packed modes.
- No segment-max subtraction: |e| <= ~8 so exp() is safe in f32/bf16,
  and the reference's emax subtraction cancels exactly.
"""

import sys
import time
import hashlib
import math

for _p in ("/opt/trn_rl_repo", "/root/.axon_site", "/root/.axon_site/_ro/trn_rl_repo"):
    if _p not in sys.path:
        sys.path.insert(0, _p)

import numpy as np
import ml_dtypes

import concourse.bass as bass
import concourse.mybir as mybir
import concourse.tile as tile
from concourse import bacc
from concourse import bass_utils

F32 = mybir.dt.float32
BF16 = mybir.dt.bfloat16
I16 = mybir.dt.int16
BF = ml_dtypes.bfloat16

NEG_SLOPE = 0.2
DENOM_EPS = 1e-30
NQ = 4  # SWDGE queues for gather round-robin


# ---------------------------------------------------------------------------
# configuration


class Cfg:
    def __init__(self, N, E, F, HID, HEADS, NC=8, GBLK=2):
        self.N, self.E, self.F, self.HID, self.HEADS, self.NC = N, E, F, HID, HEADS, NC
        self.HALF = N // 2
        self.NB = N // NC                      # nodes per core
        self.NBLK = (self.NB + 127) // 128     # dst blocks per core
        self.NBpad = self.NBLK * 128
        self.GBLK = GBLK                       # blocks per edge group
        self.KT1 = (F + 127) // 128            # GEMM1 k tiles
        self.Fpad = self.KT1 * 128
        self.C1 = HEADS * HID                  # layer-1 feature width (512)
        self.KT2 = (self.C1 + 127) // 128
        self.C1pad = self.KT2 * 128
        self.C2 = HID                          # layer-2 output width
        # layer-1 message table row (gathered at 1024 B)
        self.MW = self.C1
        # layer-1 attn window row: [el H | er H | pad] -> 128 (256 B gather)
        self.AW = 128
        self.W1OUT = self.MW + self.AW         # GEMM1 extended output width
        # layer-2 table row: [msg C2 | el 1 | er 1 | pad] -> 128
        self.RW2 = ((self.C2 + 2 + 127) // 128) * 128
        assert self.RW2 == 128
        assert self.HALF < 32768 and self.NB < 32768


# ---------------------------------------------------------------------------
# host-side edge preprocessing


def wrap_idxs(idx: np.ndarray) -> np.ndarray:
    """int16 idx list (len = nt*128) -> [128, nt*8] wrapped-16, replicated."""
    n = len(idx)
    assert n % 128 == 0
    m = n // 16
    out = np.zeros((16, m), dtype=np.int16)
    out[np.arange(n) % 16, np.arange(n) // 16] = idx
    return np.tile(out, (8, 1))


def make_plan(src, dst, cfg: Cfg):
    """Static tile plan (shared across cores) + per-core edge tensors."""
    NC, NB, NBLK, HALF, GBLK = cfg.NC, cfg.NB, cfg.NBLK, cfg.HALF, cfg.GBLK

    per_core = []
    counts = np.zeros((NC, NBLK, 2), dtype=np.int64)
    for r in range(NC):
        sel = np.nonzero((dst >= r * NB) & (dst < (r + 1) * NB))[0]
        es = src[sel].astype(np.int64)
        ed = (dst[sel] - r * NB).astype(np.int64)
        blk = ed >> 7
        half = (es >= HALF).astype(np.int64)
        key = blk * 2 + half
        o = np.argsort(key, kind="stable")
        es, ed, key = es[o], ed[o], key[o]
        cnt = np.bincount(key, minlength=NBLK * 2)
        counts[r] = cnt.reshape(NBLK, 2)
        per_core.append((es, ed, np.concatenate([[0], np.cumsum(cnt)])))

    cmax = counts.max(axis=0)                          # [NBLK, 2]
    ntile = (cmax + 127) // 128                        # tiles per (block, half)

    groups = []
    for g0 in range(0, NBLK, GBLK):
        blocks = list(range(g0, min(g0 + GBLK, NBLK)))
        ntlo = int(sum(ntile[b, 0] for b in blocks))
        nthi = int(sum(ntile[b, 1] for b in blocks))
        nt = ntlo + nthi
        # tile -> block map, block -> (first,last) tile indices
        tile_blk = []
        for b in blocks:
            tile_blk += [b] * int(ntile[b, 0])
        for b in blocks:
            tile_blk += [b] * int(ntile[b, 1])
        groups.append(dict(blocks=blocks, ntlo=ntlo, nthi=nthi, nt=nt,
                           tile_blk=tile_blk))

    # per-core flat arrays in group/tile order
    core_data = []
    for r in range(NC):
        es, ed, cum = per_core[r]
        lo_idx, hi_idx, er_idx, dvals = [], [], [], []
        for g in groups:
            for h, acc in ((0, lo_idx), (1, hi_idx)):
                for b in g["blocks"]:
                    k = b * 2 + h
                    s, e = cum[k], cum[k + 1]
                    n_pad = int(ntile[b, h]) * 128
                    v = es[s:e] - (HALF if h else 0)
                    v = np.concatenate([v, np.zeros(n_pad - len(v), np.int64)])
                    acc.append(v)
                    d = np.concatenate([ed[s:e] ,
                                        np.full(n_pad - (e - s), -1, np.int64)])
                    er_idx.append(np.maximum(d, 0))
                    dv = np.where(d >= 0, d & 127, 999).astype(np.float32)
                    dvals.append(dv)
        lo = np.concatenate(lo_idx) if lo_idx else np.zeros(0, np.int64)
        hi = np.concatenate(hi_idx) if hi_idx else np.zeros(0, np.int64)
        # NOTE er/dvals follow per-group [lo-tiles, hi-tiles] order already
        # because the loop above appends lo halves then hi halves per group.
        er = np.concatenate(er_idx)
        dv = np.concatenate(dvals)
        # wrap per group
        glo, ghi, ger, gdl = [], [], [], []
        plo = phi = pall = 0
        for g in groups:
            nlo, nhi, nt = g["ntlo"] * 128, g["nthi"] * 128, g["nt"] * 128
            glo.append(wrap_idxs(lo[plo:plo + nlo].astype(np.int16)))
            ghi.append(wrap_idxs(hi[phi:phi + nhi].astype(np.int16)))
            ger.append(wrap_idxs(er[pall:pall + nt].astype(np.int16)))
            gdl.append(dv[pall:pall + nt].reshape(g["nt"], 128).T)
            plo, phi, pall = plo + nlo, phi + nhi, pall + nt
        core_data.append(dict(
            g1lo=np.concatenate(glo, axis=1) if glo else np.zeros((128, 0), np.int16),
            g1hi=np.concatenate(ghi, axis=1) if ghi else np.zeros((128, 0), np.int16),
            ger=np.concatenate(ger, axis=1),
            dstloc=np.concatenate(gdl, axis=1).astype(BF),
        ))

    plan = dict(groups=groups, ntile=ntile,
                TLO=int(sum(g["ntlo"] for g in groups)),
                THI=int(sum(g["nthi"] for g in groups)),
                TT=int(sum(g["nt"] for g in groups)))
    return plan, core_data



MAX_GIDX = 1024  # HW limit: dma_gather crashes above 1024 indices

_GQ = {"n": 1, "i": 0}


def chunked_gather(nc, out_slab, in_ap, idx_tile, t0, ntc, elem, step=None):
    """Emit dma_gathers of <=MAX_GIDX idxs covering tiles [t0, t0+ntc) of
    out_slab; idx_tile columns are the wrapped idx layout for those tiles.
    Gathers round-robin over _GQ['n'] SWDGE queues."""
    CT = MAX_GIDX // 128
    for q0 in range(0, ntc, CT):
        qn = min(CT, ntc - q0)
        kw = dict(elem_step=step) if step else {}
        if _GQ["n"] > 1:
            kw["queue_num"] = _GQ["i"] % _GQ["n"]
            _GQ["i"] += 1
        nc.gpsimd.dma_gather(
            out_ap=out_slab[:, t0 + q0:t0 + q0 + qn, :], in_ap=in_ap,
            idxs_ap=idx_tile[:, q0 * 8:(q0 + qn) * 8],
            num_idxs=qn * 128, num_idxs_reg=qn * 128, elem_size=elem, **kw)


def pair_expand(nc, pool, bcast_ap, out_shape, dtype, tag):
    """Copy a pair-broadcast AP into a [.., 2] tile so downstream broadcasts
    have a packed innermost [1, 2] AP (DVE 2x/4x modes)."""
    t = pool.tile(list(out_shape), dtype, tag=tag)
    nc.vector.tensor_copy(t[:], bcast_ap)
    return t


# ---------------------------------------------------------------------------
# program builder


def build_program(cfg: Cfg, plan, reps=1, phases="ABCDEF", nq=NQ):
    ag_space = "Shared" if reps == 1 else "Local"
    c = cfg
    _GQ["n"], _GQ["i"] = nq, 0
    nc = bacc.Bacc("TRN2", target_bir_lowering=False, debug=False,
                   num_devices=c.NC, num_swdge_queues=nq)

    dt = nc.dram_tensor
    featsT = dt("featsT", [c.KT1, 128, c.NBpad], BF16, kind="ExternalInput")
    w1ext = dt("w1ext", [c.KT1, 128, c.W1OUT], BF16, kind="ExternalInput")
    w2ext = dt("w2ext", [c.KT2, 128, c.RW2], BF16, kind="ExternalInput")
    b1rep = dt("b1rep", [128, c.C1], F32, kind="ExternalInput")
    b2rep = dt("b2rep", [128, c.C2], F32, kind="ExternalInput")
    iota = dt("iota", [128, 128], BF16, kind="ExternalInput")
    g1lo = dt("g1lo", [128, max(1, plan["TLO"] * 8)], I16, kind="ExternalInput")
    g1hi = dt("g1hi", [128, max(1, plan["THI"] * 8)], I16, kind="ExternalInput")
    ger = dt("ger", [128, plan["TT"] * 8], I16, kind="ExternalInput")
    dstloc = dt("dstloc", [128, plan["TT"]], BF16, kind="ExternalInput")
    out = dt("out", [c.NB, c.C2], F32, kind="ExternalOutput")

    groups = plan["groups"]
    H = c.HEADS

    def build_onehot(sb, iosb, dl, nt, tag):
        """One-hot oh[e, t, d] = (iota[d] == dstloc[e, t]), via pair trick."""
        dlp = pair_expand(nc, sb,
                          dl[:, :, None].broadcast_to([128, nt, 2]),
                          [128, nt, 2], BF16, tag + "p")
        oh = sb.tile([128, nt, 128], BF16, tag=tag)
        nc.vector.tensor_tensor(
            out=oh[:].rearrange("p t (x y) -> p t x y", x=64),
            in0=iosb[:].rearrange("p (x y) -> p x y", x=64)[:, None]
                .broadcast_to([128, nt, 64, 2]),
            in1=dlp[:, :, None, :].broadcast_to([128, nt, 64, 2]),
            op=mybir.AluOpType.is_equal)
        return oh

    with tile.TileContext(nc) as tc:
        with tc.tile_pool(name="dram", bufs=1, space="DRAM") as dram:
            bmsg = dram.tile([c.NB, c.MW], BF16)
            batt = dram.tile([c.NB, c.AW], BF16)
            hmsg = dram.tile([c.N, c.MW], BF16, addr_space=ag_space)
            hatt = dram.tile([c.N, c.AW], BF16, addr_space=ag_space)
            x2d = dram.tile([c.NBpad, c.C1pad], BF16)
            bounce_h2 = dram.tile([c.NB, c.RW2], BF16)
            h2full = dram.tile([c.N, c.RW2], BF16, addr_space=ag_space)

            for _rep in range(reps):
                if "A" in phases:
                    # ---------------- phase A: GEMM1 -> bmsg/batt ----------------
                    with (
                        tc.tile_pool(name="ga", bufs=1) as cpool,
                        tc.tile_pool(name="gaw", bufs=3) as wpool,
                        tc.tile_pool(name="gap", bufs=3, space="PSUM") as pspool,
                    ):
                        w1sb = cpool.tile([128, c.KT1, c.W1OUT], BF16)
                        ftsb = cpool.tile([128, c.KT1, c.NBpad], BF16)
                        nc.sync.dma_start(w1sb[:], w1ext[:].rearrange("k p w -> p k w"))
                        nc.sync.dma_start(ftsb[:], featsT[:].rearrange("k p w -> p k w"))
                        for ntb in range(c.NBLK):
                            pa = pspool.tile([128, c.MW], F32, tag="pa", space="PSUM")
                            pb = pspool.tile([128, 16], F32, tag="pb", space="PSUM")
                            for k in range(c.KT1):
                                lhsT = ftsb[:, k, ntb * 128:(ntb + 1) * 128]
                                nc.tensor.matmul(pa[:], lhsT, w1sb[:, k, 0:c.MW],
                                                 start=(k == 0), stop=(k == c.KT1 - 1))
                                nc.tensor.matmul(pb[:], lhsT,
                                                 w1sb[:, k, c.MW:c.MW + 16],
                                                 start=(k == 0), stop=(k == c.KT1 - 1))
                            msgt = wpool.tile([128, c.MW], BF16, tag="msgt")
                            attt = wpool.tile([128, c.AW], BF16, tag="attt")
                            nc.vector.tensor_copy(msgt[:], pa[:])
                            nc.vector.memset(attt[:, 16:c.AW], 0.0)
                            nc.vector.tensor_copy(attt[:, 0:16], pb[:])
                            rows = min(128, c.NB - ntb * 128)
                            nc.sync.dma_start(bmsg[ntb * 128:ntb * 128 + rows, :],
                                              msgt[:rows, :])
                            nc.sync.dma_start(batt[ntb * 128:ntb * 128 + rows, :],
                                              attt[:rows, :])

                if "B" in phases:
                    # ---------------- phase B: AllGather hmsg/hatt ----------------
                    nc.gpsimd.collective_compute(
                        "AllGather", mybir.AluOpType.bypass,
                        replica_groups=[list(range(c.NC))],
                        ins=[bmsg.opt()], outs=[hmsg.opt()],
                    )
                    nc.gpsimd.collective_compute(
                        "AllGather", mybir.AluOpType.bypass,
                        replica_groups=[list(range(c.NC))],
                        ins=[batt.opt()], outs=[hatt.opt()],
                    )

                if "C" in phases:
                    # ---------------- phase C: layer-1 edge phase ----------------
                    with (
                        tc.tile_pool(name="ec", bufs=1) as cst,
                        tc.tile_pool(name="e1", bufs=2) as sb,
                        tc.tile_pool(name="e1n", bufs=3) as nsb,
                        tc.tile_pool(name="e1p", bufs=3, space="PSUM") as ps,
                    ):
                        iosb = cst.tile([128, 128], BF16)
                        nc.sync.dma_start(iosb[:], iota[:])
                        b1sb = cst.tile([128, c.C1], F32)
                        nc.sync.dma_start(b1sb[:], b1rep[:])
                        # relu(b1) rows for empty blocks
                        xb0 = cst.tile([128, c.C1], BF16)
                        nc.scalar.activation(xb0[:], b1sb[:],
                                             mybir.ActivationFunctionType.Relu)

                        olo = ohi = oall = 0
                        for g in groups:
                            ntlo, nthi, nt = g["ntlo"], g["nthi"], g["nt"]
                            if nt == 0:
                                continue
                            slab = sb.tile([128, nt, c.MW], BF16, tag="slab")
                            elw = sb.tile([128, nt, c.AW], BF16, tag="elw")
                            erw = sb.tile([128, nt, c.AW], BF16, tag="erw")
                            dl = sb.tile([128, nt], BF16, tag="dl")
                            nc.sync.dma_start(dl[:], dstloc[:, oall:oall + nt])
                            if ntlo:
                                ilo = sb.tile([128, ntlo * 8], I16, tag="ilo")
                                nc.sync.dma_start(ilo[:], g1lo[:, olo * 8:(olo + ntlo) * 8])
                                chunked_gather(nc, slab, hmsg[0:c.HALF, :], ilo,
                                               0, ntlo, c.MW)
                                chunked_gather(nc, elw, hatt[0:c.HALF, :], ilo,
                                               0, ntlo, c.AW)
                            if nthi:
                                ihi = sb.tile([128, nthi * 8], I16, tag="ihi")
                                nc.sync.dma_start(ihi[:], g1hi[:, ohi * 8:(ohi + nthi) * 8])
                                chunked_gather(nc, slab, hmsg[c.HALF:c.N, :], ihi,
                                               ntlo, nthi, c.MW)
                                chunked_gather(nc, elw, hatt[c.HALF:c.N, :], ihi,
                                               ntlo, nthi, c.AW)
                            ier = sb.tile([128, nt * 8], I16, tag="ier")
                            nc.sync.dma_start(ier[:], ger[:, oall * 8:(oall + nt) * 8])
                            chunked_gather(nc, erw, batt[:], ier, 0, nt, c.AW)
                            oh = build_onehot(sb, iosb, dl, nt, "oh")
                            # e = el + er ; lrelu ; exp -> a
                            et = sb.tile([128, nt, H], F32, tag="et")
                            e2 = sb.tile([128, nt, H], F32, tag="e2")
                            nc.vector.tensor_tensor(
                                out=et[:], in0=elw[:, :, 0:H],
                                in1=erw[:, :, H:2 * H],
                                op=mybir.AluOpType.add)
                            nc.vector.tensor_scalar_mul(e2[:], et[:], NEG_SLOPE)
                            nc.vector.tensor_tensor(out=e2[:], in0=e2[:], in1=et[:],
                                                    op=mybir.AluOpType.max)
                            atile = sb.tile([128, nt, H], BF16, tag="at")
                            nc.scalar.activation(atile[:], e2[:],
                                                 mybir.ActivationFunctionType.Exp)
                            apair = pair_expand(
                                nc, sb,
                                atile[:, :, :, None].broadcast_to([128, nt, H, 2]),
                                [128, nt, H, 2], BF16, "ap")
                            # msg = h * a (in-place, pair-broadcast a over HID)
                            nc.vector.tensor_tensor(
                                out=slab[:].rearrange(
                                    "p t (h x y) -> p t h x y", h=H, x=c.HID // 2),
                                in0=slab[:].rearrange(
                                    "p t (h x y) -> p t h x y", h=H, x=c.HID // 2),
                                in1=apair[:, :, :, None, :].broadcast_to(
                                    [128, nt, H, c.HID // 2, 2]),
                                op=mybir.AluOpType.mult)
                            # per-block accumulate + normalize
                            for b in g["blocks"]:
                                tlist = [t for t, tb in enumerate(g["tile_blk"]) if tb == b]
                                rows = min(128, c.NB - b * 128)
                                if not tlist:
                                    nc.sync.dma_start(
                                        x2d[b * 128:b * 128 + rows, 0:c.C1], xb0[:rows, :])
                                    continue
                                pa = ps.tile([128, c.MW], F32, tag="cpa", space="PSUM")
                                pd = ps.tile([128, H], F32, tag="cpd", space="PSUM")
                                for j, t in enumerate(tlist):
                                    st, sp = (j == 0), (j == len(tlist) - 1)
                                    nc.tensor.matmul(pa[:], oh[:, t, :],
                                                     slab[:, t, :], start=st, stop=sp)
                                    nc.tensor.matmul(pd[:], oh[:, t, :],
                                                     atile[:, t, :], start=st, stop=sp)
                                dg = nsb.tile([128, H], F32, tag="dg")
                                rd = nsb.tile([128, H], F32, tag="rd")
                                nc.vector.tensor_scalar_max(dg[:], pd[:], DENOM_EPS)
                                nc.vector.reciprocal(rd[:], dg[:])
                                rdp = pair_expand(
                                    nc, nsb,
                                    rd[:, :, None].broadcast_to([128, H, 2]),
                                    [128, H, 2], F32, "rdp")
                                xt = nsb.tile([128, c.C1], F32, tag="xt")
                                nc.vector.tensor_tensor(
                                    out=xt[:].rearrange("p (h x y) -> p h x y",
                                                        h=H, x=c.HID // 2),
                                    in0=pa[:].rearrange("p (h x y) -> p h x y",
                                                        h=H, x=c.HID // 2),
                                    in1=rdp[:, :, None, :].broadcast_to(
                                        [128, H, c.HID // 2, 2]),
                                    op=mybir.AluOpType.mult)
                                nc.vector.tensor_tensor(out=xt[:], in0=xt[:], in1=b1sb[:],
                                                        op=mybir.AluOpType.add)
                                xb = nsb.tile([128, c.C1], BF16, tag="xb")
                                nc.scalar.activation(xb[:], xt[:],
                                                     mybir.ActivationFunctionType.Relu)
                                nc.sync.dma_start(
                                    x2d[b * 128:b * 128 + rows, 0:c.C1], xb[:rows, :])
                            olo, ohi, oall = olo + ntlo, ohi + nthi, oall + nt

                if "D" in phases:
                    # ---------------- phase D: GEMM2 -> bounce_h2 ----------------
                    with (
                        tc.tile_pool(name="gb", bufs=1) as cpool,
                        tc.tile_pool(name="gbw", bufs=2) as wpool,
                        tc.tile_pool(name="gbp", bufs=2, space="PSUM") as pspool,
                    ):
                        zt = cpool.tile([128, c.C1pad], BF16)
                        nc.vector.memset(zt[:], 0.0)
                        if c.NBpad > c.NB:
                            nc.sync.dma_start(x2d[c.NB:c.NBpad, :], zt[:c.NBpad - c.NB, :])
                        if c.C1pad > c.C1:
                            # zero the pad cols (never written by phase C)
                            for ntb in range(c.NBLK):
                                nc.sync.dma_start(
                                    x2d[ntb * 128:(ntb + 1) * 128, c.C1:c.C1pad],
                                    zt[:, 0:c.C1pad - c.C1])
                        w2sb = cpool.tile([128, c.KT2, c.RW2], BF16)
                        nc.sync.dma_start(w2sb[:], w2ext[:].rearrange("k p w -> p k w"))
                        x2t = cpool.tile([128, c.KT2, c.NBpad], BF16)
                        for k in range(c.KT2):
                            kc = min(128, c.C1pad - k * 128)
                            nc.sync.dma_start_transpose(
                                x2t[0:kc, k, :], x2d[:, k * 128:k * 128 + kc])
                        for ntb in range(c.NBLK):
                            pc = pspool.tile([128, c.RW2], F32, tag="pc", space="PSUM")
                            for k in range(c.KT2):
                                nc.tensor.matmul(pc[:], x2t[:, k, ntb * 128:(ntb + 1) * 128],
                                                 w2sb[:, k, :],
                                                 start=(k == 0), stop=(k == c.KT2 - 1))
                            h2t = wpool.tile([128, c.RW2], BF16, tag="h2t")
                            nc.vector.tensor_copy(h2t[:], pc[:])
                            rows = min(128, c.NB - ntb * 128)
                            nc.sync.dma_start(bounce_h2[ntb * 128:ntb * 128 + rows, :],
                                              h2t[:rows, :])

                if "E" in phases:
                    # ---------------- phase E: AllGather h2 ----------------
                    nc.gpsimd.collective_compute(
                        "AllGather", mybir.AluOpType.bypass,
                        replica_groups=[list(range(c.NC))],
                        ins=[bounce_h2.opt()], outs=[h2full.opt()],
                    )

                if "F" in phases:
                    # ---------------- phase F: layer-2 edge phase ----------------
                    with (
                        tc.tile_pool(name="fc", bufs=1) as cst,
                        tc.tile_pool(name="f1", bufs=2) as sb,
                        tc.tile_pool(name="f1n", bufs=2) as nsb,
                        tc.tile_pool(name="f1p", bufs=8, space="PSUM") as ps,
                    ):
                        iosb = cst.tile([128, 128], BF16)
                        nc.sync.dma_start(iosb[:], iota[:])
                        b2sb = cst.tile([128, c.C2], F32)
                        nc.sync.dma_start(b2sb[:], b2rep[:])

                        olo = ohi = oall = 0
                        for g in groups:
                            ntlo, nthi, nt = g["ntlo"], g["nthi"], g["nt"]
                            if nt == 0:
                                continue
                            slab = sb.tile([128, nt, c.RW2], BF16, tag="slab2")
                            ersl = sb.tile([128, nt, c.RW2], BF16, tag="ersl2")
                            dl = sb.tile([128, nt], BF16, tag="dl2")
                            nc.sync.dma_start(dl[:], dstloc[:, oall:oall + nt])
                            if ntlo:
                                ilo = sb.tile([128, ntlo * 8], I16, tag="ilo2")
                                nc.sync.dma_start(ilo[:], g1lo[:, olo * 8:(olo + ntlo) * 8])
                                chunked_gather(nc, slab, h2full[0:c.HALF, :], ilo,
                                               0, ntlo, c.RW2)
                            if nthi:
                                ihi = sb.tile([128, nthi * 8], I16, tag="ihi2")
                                nc.sync.dma_start(ihi[:], g1hi[:, ohi * 8:(ohi + nthi) * 8])
                                chunked_gather(nc, slab, h2full[c.HALF:c.N, :], ihi,
                                               ntlo, nthi, c.RW2)
                            ier = sb.tile([128, nt * 8], I16, tag="ier2")
                            nc.sync.dma_start(ier[:], ger[:, oall * 8:(oall + nt) * 8])
                            chunked_gather(nc, ersl, bounce_h2[:], ier,
                                           0, nt, c.RW2)
                            oh = build_onehot(sb, iosb, dl, nt, "oh2")
                            et = sb.tile([128, nt, 1], F32, tag="et2")
                            e2 = sb.tile([128, nt, 1], F32, tag="e22")
                            nc.vector.tensor_tensor(
                                out=et[:], in0=slab[:, :, c.C2:c.C2 + 1],
                                in1=ersl[:, :, c.C2 + 1:c.C2 + 2], op=mybir.AluOpType.add)
                            nc.vector.tensor_scalar_mul(e2[:], et[:], NEG_SLOPE)
                            nc.vector.tensor_tensor(out=e2[:], in0=e2[:], in1=et[:],
                                                    op=mybir.AluOpType.max)
                            nc.scalar.activation(slab[:, :, c.C2:c.C2 + 1], e2[:],
                                                 mybir.ActivationFunctionType.Exp)
                            a2p = pair_expand(
                                nc, sb,
                                slab[:, :, c.C2:c.C2 + 1].broadcast_to([128, nt, 2]),
                                [128, nt, 2], BF16, "a2p")
                            nc.vector.tensor_tensor(
                                out=slab[:, :, 0:c.C2].rearrange(
                                    "p t (x y) -> p t x y", x=c.C2 // 2),
                                in0=slab[:, :, 0:c.C2].rearrange(
                                    "p t (x y) -> p t x y", x=c.C2 // 2),
                                in1=a2p[:, :, None, :].broadcast_to(
                                    [128, nt, c.C2 // 2, 2]),
                                op=mybir.AluOpType.mult)
                            for b in g["blocks"]:
                                tlist = [t for t, tb in enumerate(g["tile_blk"]) if tb == b]
                                rows = min(128, c.NB - b * 128)
                                if not tlist:
                                    ot = nsb.tile([128, c.C2], F32, tag="ot")
                                    nc.vector.tensor_copy(ot[:], b2sb[:])
                                    nc.sync.dma_start(out[b * 128:b * 128 + rows, :],
                                                      ot[:rows, :])
                                    continue
                                pc = ps.tile([128, c.C2 + 1], F32, tag="pc2", space="PSUM")
                                for j, t in enumerate(tlist):
                                    nc.tensor.matmul(pc[:], oh[:, t, :],
                                                     slab[:, t, 0:c.C2 + 1],
                                                     start=(j == 0),
                                                     stop=(j == len(tlist) - 1))
                                dg = nsb.tile([128, 1], F32, tag="dg2")
                                rd = nsb.tile([128, 1], F32, tag="rd2")
                                nc.vector.tensor_scalar_max(dg[:], pc[:, c.C2:c.C2 + 1],
                                                            DENOM_EPS)
                                nc.vector.reciprocal(rd[:], dg[:])
                                ot = nsb.tile([128, c.C2], F32, tag="ot")
                                nc.vector.tensor_scalar(
                                    out=ot[:], in0=pc[:, 0:c.C2], scalar1=rd[:, 0:1],
                                    scalar2=None, op0=mybir.AluOpType.mult)
                                nc.vector.tensor_tensor(out=ot[:], in0=ot[:], in1=b2sb[:],
                                                        op=mybir.AluOpType.add)
                                nc.sync.dma_start(out[b * 128:b * 128 + rows, :],
                                                  ot[:rows, :])
                            olo, ohi, oall = olo + ntlo, ohi + nthi, oall + nt

    nc.compile()
    return nc

# ---------------------------------------------------------------------------
# host orchestration


def make_inputs(inputs, cfg: Cfg, plan, core_data):
    c = cfg
    feats = np.asarray(inputs["feats"], np.float32)
    W1 = np.asarray(inputs["W1"], np.float32)
    al1 = np.asarray(inputs["attn_l1"], np.float32)
    ar1 = np.asarray(inputs["attn_r1"], np.float32)
    b1 = np.asarray(inputs["b1"], np.float32)
    W2 = np.asarray(inputs["W2"], np.float32)
    al2 = np.asarray(inputs["attn_l2"], np.float32)
    ar2 = np.asarray(inputs["attn_r2"], np.float32)
    b2 = np.asarray(inputs["b2"], np.float32)

    H, HID = c.HEADS, c.HID
    W1r = W1.reshape(c.F, H, HID)
    Wl1 = np.einsum("khd,hd->kh", W1r, al1)
    Wr1 = np.einsum("khd,hd->kh", W1r, ar1)
    w1e = np.zeros((c.Fpad, c.W1OUT), np.float32)
    w1e[:c.F, 0:c.C1] = W1
    w1e[:c.F, c.MW:c.MW + H] = Wl1
    w1e[:c.F, c.MW + H:c.MW + 2 * H] = Wr1
    w1e = w1e.reshape(c.KT1, 128, c.W1OUT).astype(BF)

    Wl2 = W2 @ al2[0]
    Wr2 = W2 @ ar2[0]
    w2e = np.zeros((c.C1pad, c.RW2), np.float32)
    w2e[:c.C1, 0:c.C2] = W2
    w2e[:c.C1, c.C2] = Wl2
    w2e[:c.C1, c.C2 + 1] = Wr2
    w2e = w2e.reshape(c.KT2, 128, c.RW2).astype(BF)

    b1r = np.tile(b1[None, :], (128, 1)).astype(np.float32)
    b2r = np.tile(b2[None, :], (128, 1)).astype(np.float32)
    iot = np.tile(np.arange(128, dtype=np.float32)[None, :], (128, 1)).astype(BF)

    in_maps = []
    for r in range(c.NC):
        ft = np.zeros((c.Fpad, c.NBpad), np.float32)
        ft[:c.F, :c.NB] = feats[r * c.NB:(r + 1) * c.NB].T
        cd = core_data[r]
        in_maps.append(dict(
            featsT=ft.reshape(c.KT1, 128, c.NBpad).astype(BF),
            w1ext=w1e, w2ext=w2e, b1rep=b1r, b2rep=b2r, iota=iot,
            g1lo=cd["g1lo"] if cd["g1lo"].shape[1] else
                np.zeros((128, 1), np.int16),
            g1hi=cd["g1hi"] if cd["g1hi"].shape[1] else
                np.zeros((128, 1), np.int16),
            ger=cd["ger"], dstloc=cd["dstloc"],
        ))
    return in_maps


_CACHE = {}


def _get_compiled(inputs, cfg):
    src = np.asarray(inputs["src"], np.int64)
    dst = np.asarray(inputs["dst"], np.int64)
    key = hashlib.sha1(np.ascontiguousarray(src).tobytes()
                       + np.ascontiguousarray(dst).tobytes()).hexdigest()
    if key not in _CACHE:
        plan, core_data = make_plan(src, dst, cfg)
        nc = build_program(cfg, plan)
        _CACHE[key] = (nc, plan, core_data)
    return _CACHE[key]


def kernel(**inputs) -> np.ndarray:
    feats = np.asarray(inputs["feats"])
    H, HID = np.asarray(inputs["attn_l1"]).shape
    cfg = Cfg(N=feats.shape[0], E=np.asarray(inputs["src"]).shape[0],
              F=feats.shape[1], HID=HID, HEADS=H)
    nc, plan, core_data = _get_compiled(inputs, cfg)
    in_maps = make_inputs(inputs, cfg, plan, core_data)
    res = bass_utils.run_bass_kernel_spmd(
        nc, in_maps, core_ids=list(range(cfg.NC)), trace=False)
    return np.concatenate([res.results[r]["out"] for r in range(cfg.NC)], axis=0)


# ---------------------------------------------------------------------------
# device-resident timing runner


class Runner:
    """Compiled SPMD executable with device-resident inputs.

    No donation: the kernel writes every output element, so uninitialized
    custom-call results are fine and the zero buffers stay reusable.
    """

    def __init__(self, nc, in_maps, n_cores):
        import jax
        from jax.experimental.shard_map import shard_map
        from jax.sharding import Mesh, PartitionSpec
        from concourse import bass2jax, mybir as mb

        bass2jax.install_neuronx_cc_hook()
        pid_name = (nc.partition_id_tensor.name
                    if nc.partition_id_tensor else None)
        in_names, out_names, out_avals, zero_outs = [], [], [], []
        for alloc in nc.m.functions[0].allocations:
            if not isinstance(alloc, mb.MemoryLocationSet):
                continue
            name = alloc.memorylocations[0].name
            if alloc.kind == "ExternalInput":
                if name != pid_name:
                    in_names.append(name)
            elif alloc.kind == "ExternalOutput":
                out_names.append(name)
                out_avals.append(jax.core.ShapedArray(
                    tuple(alloc.tensor_shape), mb.dt.np(alloc.dtype)))
                zero_outs.append(np.zeros(alloc.tensor_shape,
                                          mb.dt.np(alloc.dtype)))
        n_params = len(in_names)
        all_names = in_names + out_names

        if pid_name is not None:
            all_names = all_names + [pid_name]

        def _body(*args):
            operands = list(args)
            if pid_name is not None:
                operands.append(bass2jax.partition_id_tensor())
            outs = bass2jax._bass_exec_p.bind(
                *operands, out_avals=tuple(out_avals), in_names=tuple(all_names),
                out_names=tuple(out_names), lowering_input_output_aliases=(),
                sim_require_finite=True, sim_require_nnan=True, nc=nc)
            return tuple(outs)

        devices = jax.devices()[:n_cores]
        mesh = Mesh(np.asarray(devices), ("core",))
        specs = (PartitionSpec("core"),) * (n_params + len(out_names))
        self._fn = jax.jit(shard_map(_body, mesh=mesh, in_specs=specs,
                                     out_specs=(PartitionSpec("core"),) * len(out_names),
                                     check_rep=False), keep_unused=True)
        concat_in = [np.concatenate([np.asarray(in_maps[c][nm])
                                     for c in range(n_cores)], axis=0)
                     for nm in in_names]
        concat_zero = [np.zeros((n_cores * z.shape[0], *z.shape[1:]), z.dtype)
                       for z in zero_outs]
        self._args = [jax.device_put(a) for a in concat_in + concat_zero]
        self.out_names, self.out_avals, self.n_cores = out_names, out_avals, n_cores

    def run(self):
        outs = self._fn(*self._args)
        for o in outs:
            o.block_until_ready()
        return outs

    def results(self):
        import numpy as _np
        outs = self.run()
        return [
            {nm: _np.asarray(outs[i]).reshape(self.n_cores,
                                              *self.out_avals[i].shape)[c]
             for i, nm in enumerate(self.out_names)}
            for c in range(self.n_cores)
        ]

    def time_ns(self, iters=12, warmup=3):
        for _ in range(warmup):
            self.run()
        best = float("inf")
        for _ in range(iters):
            t0 = time.perf_counter()
            self.run()
            best = min(best, time.perf_counter() - t0)
        return best * 1e9

    def time_once_ns(self):
        t0 = time.perf_counter()
        self.run()
        return (time.perf_counter() - t0) * 1e9

    def time_batch_ns(self, k):
        """Launch k executions async, block once; amortizes dispatch jitter."""
        t0 = time.perf_counter()
        outs = None
        for _ in range(k):
            outs = self._fn(*self._args)
        for o in outs:
            o.block_until_ready()
        return (time.perf_counter() - t0) * 1e9


def measure_hw_ns(inputs, reps_hi=9, phases="ABCDEF", iters=12, nq=NQ):
    """Device time per kernel via repeat-delta: (t[R] - t[1]) / (R - 1)."""
    feats = np.asarray(inputs["feats"])
    H, HID = np.asarray(inputs["attn_l1"]).shape
    cfg = Cfg(N=feats.shape[0], E=np.asarray(inputs["src"]).shape[0],
              F=feats.shape[1], HID=HID, HEADS=H)
    src = np.asarray(inputs["src"], np.int64)
    dst = np.asarray(inputs["dst"], np.int64)
    plan, core_data = make_plan(src, dst, cfg)
    in_maps = make_inputs(inputs, cfg, plan, core_data)
    runners = {}
    for reps in (1, reps_hi):
        nc = build_program(cfg, plan, reps=reps, phases=phases, nq=nq)
        runners[reps] = Runner(nc, in_maps, cfg.NC)
        for _ in range(2):
            runners[reps].run()
    # interleave timing rounds so co-tenant noise hits both builds alike
    t = {1: float("inf"), reps_hi: float("inf")}
    for _ in range(iters):
        for reps in (1, reps_hi):
            t[reps] = min(t[reps], runners[reps].time_once_ns())
    del runners
    return (t[reps_hi] - t[1]) / (reps_hi - 1)
